# revision 28
# baseline (speedup 1.0000x reference)
"""Trainium2 Bass kernel for the CAM factorized-attention module.

Reference computation (per batch element b, C=256, N=P*H*W=12288, h=8 heads,
Ch=32):
    x1   = x[b].reshape(C, N).T                      # [N, C]
    qkv  = x1 @ W_qkv + b_qkv                        # [N, 3C]
    q, k, v  (each [h, N, Ch])
    kw   = softmax(k, axis=N)
    kv   = kw^T @ v (per head)                       # [h, Ch, Ch]
    fa   = q @ kv                                    # [h, N, Ch]
    out  = (scale * fa).reshape(N, C) @ W_proj + b_proj
    res  = gamma * out.T.reshape(C, P, H, W) + x[b]

Sharding: data-parallel over B — core i computes batch element i, no
collectives.

Precision plan: the attention branch is ~0.3% of the output magnitude
(output = x + gamma*attn with |gamma*attn| tiny), so the branch tolerates
aggressive quantization.  The two large matmul families (k/v projection and
the final collapsed M @ x) run in fp8e4 DoubleRow mode (2 MACs/cell/cycle,
contraction 256 in one pass); everything downstream of the softmax
(kv, fold) runs bf16 with fp32 PSUM accumulation; the residual x is added in
exact fp32.  End-to-end error vs the fp32 reference is ~2e-5 (CPU-verified).

Algebraic restructuring (exact up to rounding):
  * k bias cancels in softmax (constant along the softmax axis)  -> dropped.
  * no max-subtraction needed (|k| < ~4); the softmax denominator is applied
    to the tiny per-head [Ch, Ch] kv matrix, not the [N, C] weight field.
    Denominators come free as an extra ones column in the kv matmul.
  * v bias folds into kv:  kv_true = (E^T v_raw)/S + b_v (row vec).
  * scale & gamma fold into W_proj;  gamma folds into b_proj (host side).
  * q is never materialized, and once kv is known the whole branch collapses
    to ONE linear map of x:
        attn^T = M^T x + bias_eff 1^T
        M[kc][mt]  = sum_t  Wq[kc,tblk] @ kvblk[t] @ Wp'[tblk,mtblk]
        bias_eff   = sum_t  Wp'[tblk,mtblk]^T kvblk[t]^T bq[tblk] + bp'
    M ([256,256] total) is fused on-chip with 14 tiny matmuls after the kv
    accumulation finishes, scaled by 2^17 into fp8e4 range (entries are
    ~1e-4; the 2^-17 unscale rides the ACT epilogue's scale parameter).

Per-core pipeline:
  load x8 (fp8, [ki,ko,n] with c = ko*128+ki), wkv8; xf (fp32) streams in
  phase 1 (48 pairs of 128-token chunks):
    k||v = x8^T wkv8  (one DoubleRow matmul per chunk, PSUM [128,1024]/pair)
    E = exp(k) (one ACT op per pair);  vb = [v|1] bf16 (one DVE copy/chunk)
    kvps[pi%2] += E_half^T vb_half  (two parity-alternating PSUM tiles)
  finalize: kvsum = kvps[0]+kvps[1];  kvblk = diag(kvsum)/S + bv  (bf16)
  fold: G' = kvblk^T Wq^T;  M8 = 2^17 * G'^T Wp' (fp8);  bias_eff
  phase 2 (24 chunks of 512):  pp = M8^T x8  (one DoubleRow matmul per mt);
    tmp = pp*2^-17 + bias_eff (ACT);  osb = tmp + xf (DVE/GPSIMD);  DMA out
"""

import sys

sys.path.insert(0, "/opt/trn_rl_repo")

import numpy as np
import ml_dtypes

import concourse.bacc as bacc
import concourse.mybir as mybir
from concourse.tile import TileContext
from concourse.bass_utils import run_bass_kernel_spmd

FP32 = mybir.dt.float32
BF16 = mybir.dt.bfloat16
FP8 = mybir.dt.float8e4
AF = mybir.ActivationFunctionType
DR = mybir.MatmulPerfMode.DoubleRow

C = 256
N = 12288
NCORES = 8
NPAIR = N // 256  # 48 pairs of 128-token chunks
NJUMBO = N // 512  # 24 chunks of 512 tokens
NPIECE = 4  # xf load granularity
M_SCALE = 131072.0  # 2^17

_CACHE = {}


def _build_nc(debug=False):
    from concourse.alu_op_type import AluOpType

    nc = bacc.Bacc(trn_type="TRN2", target_bir_lowering=False)

    x8_d = nc.declare_dram_parameter("x8", [128, 2, N], FP8, False)
    xf_d = nc.declare_dram_parameter("xf", [2, 128, N], FP32, False)
    wkv8_d = nc.declare_dram_parameter("wkv8", [128, 2, 512], FP8, False)
    wqt_d = nc.declare_dram_parameter("wqt", [2, 128, 256], BF16, False)
    wp_d = nc.declare_dram_parameter("wp", [2, 128, 256], BF16, False)
    bq_d = nc.declare_dram_parameter("bq", [2, 128, 1], BF16, False)
    bp_d = nc.declare_dram_parameter("bp", [2, 128, 1], FP32, False)
    bv_d = nc.declare_dram_parameter("bv", [2, 128, 32], FP32, False)
    out_d = nc.declare_dram_parameter("out", [2, 128, N], FP32, True)
    if debug:
        dbg_kvps = nc.declare_dram_parameter("dbg_kvps", [2, 128, 129], FP32, True)
        dbg_kvblk = nc.declare_dram_parameter("dbg_kvblk", [2, 128, 128], BF16, True)
        dbg_be = nc.declare_dram_parameter("dbg_be", [2, 128, 1], FP32, True)

    PIECE = N // NPIECE

    with TileContext(nc) as tc:
        with (
            tc.tile_pool(name="const", bufs=1) as const,
            tc.tile_pool(name="resident", bufs=1) as resident,
        ):
            # --- resident tensors -------------------------------------------
            x8 = resident.tile([128, 2, N], FP8, name="x8")
            xf = [resident.tile([128, N], FP32, name=f"xf{t}") for t in range(2)]
            wkv8 = const.tile([128, 2, 512], FP8, name="wkv8")
            wqt = [const.tile([128, 256], BF16, name=f"wqt{t}") for t in range(2)]
            wp = [const.tile([128, 256], BF16, name=f"wp{t}") for t in range(2)]
            bq = [const.tile([128, 1], BF16, name=f"bq{t}") for t in range(2)]
            bp = [const.tile([128, 1], FP32, name=f"bp{t}") for t in range(2)]
            bv = [const.tile([128, 32], FP32, name=f"bv{t}") for t in range(2)]
            kvblk = [const.tile([128, 128], BF16, name=f"kvblk{t}") for t in range(2)]
            Gp = [
                [const.tile([128, 128], BF16, name=f"Gp{t}{kc}") for kc in range(2)]
                for t in range(2)
            ]
            M8 = [const.tile([128, 2, 128], FP8, name=f"M8{mt}") for mt in range(2)]
            cq = [const.tile([128, 1], BF16, name=f"cq{t}") for t in range(2)]
            be = [const.tile([128, 1], FP32, name=f"be{mt}") for mt in range(2)]
            recip = [const.tile([128, 1], FP32, name=f"recip{t}") for t in range(2)]
            vb = [const.tile([128, 258], BF16, name=f"vb{j}") for j in range(6)]
            kvsum = const.tile([128, 258], FP32, name="kvsum")

            # phase-1 gates first: x8 + wkv8
            nc.sync.dma_start(x8[:], x8_d[:, :, :])
            nc.sync.dma_start(wkv8[:], wkv8_d[:, :, :])
            for t in range(2):
                nc.sync.dma_start(wqt[t][:], wqt_d[t])
                nc.sync.dma_start(wp[t][:], wp_d[t])
                nc.sync.dma_start(bq[t][:], bq_d[t])
                nc.sync.dma_start(bp[t][:], bp_d[t])
                nc.sync.dma_start(bv[t][:], bv_d[t])
                nc.vector.memset(kvblk[t][:], 0.0)
            for j in range(6):
                nc.vector.memset(
                    vb[j][:].rearrange("p (s x) -> p s x", x=129)[:, :, 128:129], 1.0
                )
            # xf only matters from phase 2 on; stream it during phase 1
            for i in range(NPIECE):
                for t in range(2):
                    nc.sync.dma_start(
                        xf[t][:, i * PIECE : (i + 1) * PIECE],
                        xf_d[t, :, i * PIECE : (i + 1) * PIECE],
                    )

            # --- phase 1: k||v, exp, kv accumulation ------------------------
            with (
                tc.tile_pool(name="p1ps", bufs=1, space="PSUM") as p1ps,
                tc.tile_pool(name="kvp_ps", bufs=3, space="PSUM") as kvp_ps,
                tc.tile_pool(name="ework", bufs=4) as ework,
            ):
                # two parity-alternating accumulators (t0 at cols 0:129, t1 at
                # 129:258) so consecutive pairs' kv matmuls are independent
                kvps = [
                    p1ps.tile([128, 258], FP32, name=f"kvps{par}") for par in range(2)
                ]

                for pi in range(NPAIR):
                    par = pi % 2
                    first, last = pi < 2, pi >= NPAIR - 2
                    kvp = kvp_ps.tile([128, 1024], FP32, name="kvp", tag="kvp")
                    for half in range(2):
                        n0 = (pi * 2 + half) * 128
                        f0 = half * 512
                        nc.tensor.matmul(
                            kvp[:, f0 : f0 + 512],
                            lhsT=x8[:, :, n0 : n0 + 128], rhs=wkv8[:],
                            start=True, stop=True, perf_mode=DR,
                        )
                    # one exp over both chunks' k columns (strided view)
                    E = ework.tile([128, 512], BF16, name="E", tag="E")
                    nc.scalar.activation(
                        E[:].rearrange("p (s x) -> p s x", x=256),
                        kvp[:].rearrange("p (s x) -> p s x", x=512)[:, :, 0:256],
                        AF.Exp,
                    )
                    for half in range(2):
                        f0 = half * 512
                        v = vb[(pi * 2 + half) % 6]
                        nc.vector.tensor_copy(
                            v[:].rearrange("p (s x) -> p s x", x=129)[:, :, 0:128],
                            kvp[:, f0 + 256 : f0 + 512].rearrange(
                                "p (s x) -> p s x", x=128
                            ),
                        )
                        for t in range(2):
                            nc.tensor.matmul(
                                kvps[par][:, t * 129 : t * 129 + 129],
                                lhsT=E[
                                    :,
                                    half * 256 + t * 128 : half * 256 + t * 128 + 128,
                                ],
                                rhs=v[:, t * 129 : t * 129 + 129],
                                start=(first and half == 0),
                                stop=(last and half == 1),
                                skip_group_check=True,
                            )

                # --- finalize kv: merge parities, normalize, add v bias -----
                nc.vector.tensor_copy(kvsum[:], kvps[0][:])
                nc.vector.tensor_add(kvsum[:], kvsum[:], kvps[1][:])
                if debug:
                    for t in range(2):
                        nc.sync.dma_start(
                            dbg_kvps[t], kvsum[:, t * 129 : t * 129 + 129]
                        )
                for t in range(2):
                    c0 = t * 129
                    nc.vector.reciprocal(recip[t][:], kvsum[:, c0 + 128 : c0 + 129])
                    for g in range(4):
                        r0 = g * 32
                        nc.vector.scalar_tensor_tensor(
                            kvblk[t][r0 : r0 + 32, r0 : r0 + 32],
                            kvsum[r0 : r0 + 32, c0 + r0 : c0 + r0 + 32],
                            recip[t][r0 : r0 + 32, :],
                            bv[t][r0 : r0 + 32, :],
                            op0=AluOpType.mult,
                            op1=AluOpType.add,
                        )

            # --- fold: G' = kvblk^T Wq^T, M8 = 2^17 G'^T Wp', bias_eff ------
            with tc.tile_pool(name="gps", bufs=4, space="PSUM") as gps:
                for t in range(2):
                    cq_ps = gps.tile([128, 1], FP32, name=f"cqps{t}", tag="little")
                    nc.tensor.matmul(
                        cq_ps[:], lhsT=kvblk[t][:], rhs=bq[t][:],
                        start=True, stop=True,
                    )
                    nc.vector.tensor_copy(cq[t][:], cq_ps[:])
                    for kc in range(2):
                        g_ps = gps.tile([128, 128], FP32, name=f"gps{t}{kc}", tag="big")
                        nc.tensor.matmul(
                            g_ps[:],
                            lhsT=kvblk[t][:],
                            rhs=wqt[t][:, kc * 128 : kc * 128 + 128],
                            start=True, stop=True,
                        )
                        nc.vector.tensor_copy(Gp[t][kc][:], g_ps[:])
                for mt in range(2):
                    be_ps = gps.tile([128, 1], FP32, name=f"beps{mt}", tag="little")
                    for t in range(2):
                        nc.tensor.matmul(
                            be_ps[:],
                            lhsT=wp[t][:, mt * 128 : mt * 128 + 128],
                            rhs=cq[t][:],
                            start=(t == 0), stop=(t == 1),
                        )
                    nc.vector.tensor_add(be[mt][:], be_ps[:], bp[mt][:])
                    for kc in range(2):
                        m_ps = gps.tile([128, 128], FP32, name=f"mps{kc}{mt}", tag="big")
                        for t in range(2):
                            nc.tensor.matmul(
                                m_ps[:],
                                lhsT=Gp[t][kc][:],
                                rhs=wp[t][:, mt * 128 : mt * 128 + 128],
                                start=(t == 0), stop=(t == 1),
                            )
                        nc.scalar.activation(
                            M8[mt][:, kc, :], m_ps[:], AF.Identity, scale=M_SCALE
                        )
                if debug:
                    for mt in range(2):
                        nc.sync.dma_start(dbg_be[mt], be[mt][:])
                    for t in range(2):
                        nc.sync.dma_start(dbg_kvblk[t], kvblk[t][:])

            # --- phase 2: pp = M8^T x8;  out = pp/2^17 + bias_eff + xf ------
            with (
                tc.tile_pool(name="pp_ps", bufs=6, space="PSUM") as pp_ps,
                tc.tile_pool(name="p2out", bufs=10) as p2out,
            ):
                for cj in range(NJUMBO):
                    n0 = cj * 512
                    for mt in range(2):
                        pp = pp_ps.tile([128, 512], FP32, name="pp", tag="pp")
                        nc.tensor.matmul(
                            pp[:], lhsT=M8[mt][:], rhs=x8[:, :, n0 : n0 + 512],
                            start=True, stop=True, perf_mode=DR,
                        )
                        tmp = p2out.tile([128, 512], FP32, name="tmp", tag="tmp")
                        nc.scalar.activation(
                            tmp[:], pp[:], AF.Identity,
                            bias=be[mt][:], scale=1.0 / M_SCALE,
                        )
                        osb = p2out.tile([128, 512], FP32, name="osb", tag="osb")
                        if mt == 0:
                            nc.vector.tensor_add(
                                osb[:], tmp[:], xf[mt][:, n0 : n0 + 512]
                            )
                        else:
                            nc.gpsimd.tensor_add(
                                osb[:], tmp[:], xf[mt][:, n0 : n0 + 512]
                            )
                        nc.sync.dma_start(out_d[mt, :, n0 : n0 + 512], osb[:])
    nc.finalize()
    return nc


def _get_nc():
    if "nc" not in _CACHE:
        _CACHE["nc"] = _build_nc()
    return _CACHE["nc"]


def _prep_in_maps(x, W_qkv, b_qkv, W_proj, b_proj, gamma):
    bf = ml_dtypes.bfloat16
    f8 = ml_dtypes.float8_e4m3
    scale = 32 ** (-0.5)
    g = float(np.asarray(gamma).reshape(-1)[0])

    # fp8 operands use contraction index c = ko*128 + ki -> layout [ki, ko, :]
    Wkv8 = np.ascontiguousarray(
        W_qkv[:, 256:768].reshape(2, 128, 512).swapaxes(0, 1)).astype(f8)
    WqT = np.ascontiguousarray(
        W_qkv[:, 0:256].T.reshape(2, 128, 256)).astype(bf)
    Wp = np.ascontiguousarray(
        (W_proj * (scale * g)).reshape(2, 128, 256)).astype(bf)
    bq = np.ascontiguousarray(
        b_qkv[0:256].reshape(2, 128, 1)).astype(bf)
    bp = np.ascontiguousarray(
        (g * b_proj).reshape(2, 128, 1)).astype(np.float32)
    # bv[t][p, cv] = b_qkv[512 + (t*4 + p//32)*32 + cv]
    bv = np.ascontiguousarray(
        np.broadcast_to(
            b_qkv[512:768].reshape(2, 4, 1, 32), (2, 4, 32, 32)
        ).reshape(2, 128, 32)
    ).astype(np.float32)

    in_maps = []
    for b in range(NCORES):
        xb = np.ascontiguousarray(x[b].reshape(C, N))
        x8 = np.ascontiguousarray(
            xb.reshape(2, 128, N).swapaxes(0, 1)).astype(f8)
        in_maps.append(
            {
                "x8": x8,
                "xf": xb.reshape(2, 128, N),
                "wkv8": Wkv8, "wqt": WqT, "wp": Wp,
                "bq": bq, "bp": bp, "bv": bv,
            }
        )
    return in_maps


def kernel(x, W_qkv, b_qkv, W_proj, b_proj, gamma, _trace=False, _trace_kwargs=None):
    x = np.asarray(x, dtype=np.float32)
    nc = _get_nc()
    in_maps = _prep_in_maps(
        x,
        np.asarray(W_qkv, np.float32),
        np.asarray(b_qkv, np.float32),
        np.asarray(W_proj, np.float32),
        np.asarray(b_proj, np.float32),
        np.asarray(gamma, np.float32),
    )
    kw = {}
    if _trace:
        kw = {"trace": True, **(_trace_kwargs or {})}
    res = run_bass_kernel_spmd(nc, in_maps, list(range(NCORES)), **kw)
    out = np.stack(
        [res.results[b]["out"].reshape(C, 3, 64, 64) for b in range(NCORES)]
    ).astype(np.float32)
    if _trace:
        return out, res
    return out


# revision 33
# speedup vs baseline: 1.0998x; 1.0998x over previous
"""Trainium2 Bass kernel for the CAM factorized-attention module.

Reference computation (per batch element b, C=256, N=P*H*W=12288, h=8 heads,
Ch=32):
    x1   = x[b].reshape(C, N).T                      # [N, C]
    qkv  = x1 @ W_qkv + b_qkv                        # [N, 3C]
    q, k, v  (each [h, N, Ch])
    kw   = softmax(k, axis=N)
    kv   = kw^T @ v (per head)                       # [h, Ch, Ch]
    fa   = q @ kv                                    # [h, N, Ch]
    out  = (scale * fa).reshape(N, C) @ W_proj + b_proj
    res  = gamma * out.T.reshape(C, P, H, W) + x[b]

Sharding: data-parallel over B — core i computes batch element i, no
collectives.

Precision plan: the attention branch is ~0.3% of the output magnitude
(output = x + gamma*attn with |gamma*attn| tiny), so the branch tolerates
aggressive quantization.  The two large matmul families (k/v projection and
the final collapsed M @ x) run in fp8e4 DoubleRow mode (2 MACs/cell/cycle,
contraction 256 in one pass); everything downstream of the softmax
(kv, fold) runs bf16 with fp32 PSUM accumulation; the residual x is added in
exact fp32.  End-to-end error vs the fp32 reference is ~2e-5 (CPU-verified).

Algebraic restructuring (exact up to rounding):
  * k bias cancels in softmax (constant along the softmax axis)  -> dropped.
  * no max-subtraction needed (|k| < ~4); the softmax denominator is applied
    to the tiny per-head [Ch, Ch] kv matrix, not the [N, C] weight field.
    Denominators come free as an extra ones column in the kv matmul.
  * v bias folds into kv:  kv_true = (E^T v_raw)/S + b_v (row vec).
  * scale & gamma fold into W_proj;  gamma folds into b_proj (host side).
  * q is never materialized, and once kv is known the whole branch collapses
    to ONE linear map of x:
        attn^T = M^T x + bias_eff 1^T
        M[kc][mt]  = sum_t  Wq[kc,tblk] @ kvblk[t] @ Wp'[tblk,mtblk]
        bias_eff   = sum_t  Wp'[tblk,mtblk]^T kvblk[t]^T bq[tblk] + bp'
    M ([256,256] total) is fused on-chip with 14 tiny matmuls after the kv
    accumulation finishes, scaled by 2^17 into fp8e4 range (entries are
    ~1e-4; the 2^-17 unscale rides the ACT epilogue's scale parameter).

Per-core pipeline:
  load x8 (fp8, [ki,ko,n] with c = ko*128+ki), wkv8; xf (fp32) streams in
  phase 1 (48 pairs of 128-token chunks):
    k||v = x8^T wkv8  (one DoubleRow matmul per chunk, PSUM [128,1024]/pair)
    E = exp(k) (one ACT op per pair);  vb = [v|1] bf16 (one DVE copy/chunk)
    kvps[pi%2] += E_half^T vb_half  (two parity-alternating PSUM tiles)
  finalize: kvsum = kvps[0]+kvps[1];  kvblk = diag(kvsum)/S + bv  (bf16)
  fold: G' = kvblk^T Wq^T;  M8 = 2^17 * G'^T Wp' (fp8);  bias_eff
  phase 2 (24 chunks of 512):  pp = M8^T x8  (one DoubleRow matmul per mt);
    tmp = pp*2^-17 + bias_eff (ACT);  osb = tmp + xf (DVE/GPSIMD);  DMA out
"""

import sys

sys.path.insert(0, "/opt/trn_rl_repo")

import numpy as np
import ml_dtypes

import concourse.bacc as bacc
import concourse.mybir as mybir
from concourse.tile import TileContext
from concourse.bass_utils import run_bass_kernel_spmd

FP32 = mybir.dt.float32
BF16 = mybir.dt.bfloat16
FP8 = mybir.dt.float8e4
AF = mybir.ActivationFunctionType
DR = mybir.MatmulPerfMode.DoubleRow

C = 256
N = 12288
NCORES = 8
NPAIR = N // 256  # 48 pairs of 128-token chunks
NJUMBO = N // 512  # 24 chunks of 512 tokens
NPIECE = 4  # xf load granularity
M_SCALE = 131072.0  # 2^17

_CACHE = {}


def _build_nc(debug=False):
    from concourse.alu_op_type import AluOpType

    nc = bacc.Bacc(trn_type="TRN2", target_bir_lowering=False)

    x8_d = nc.declare_dram_parameter("x8", [128, 2, N], FP8, False)
    xf_d = nc.declare_dram_parameter("xf", [2, 128, N], FP32, False)
    wkv8_d = nc.declare_dram_parameter("wkv8", [128, 2, 512], FP8, False)
    wqt_d = nc.declare_dram_parameter("wqt", [2, 128, 256], BF16, False)
    wp_d = nc.declare_dram_parameter("wp", [2, 128, 256], BF16, False)
    bq_d = nc.declare_dram_parameter("bq", [2, 128, 1], BF16, False)
    bp_d = nc.declare_dram_parameter("bp", [2, 128, 1], FP32, False)
    bv_d = nc.declare_dram_parameter("bv", [2, 128, 32], FP32, False)
    out_d = nc.declare_dram_parameter("out", [2, 128, N], FP32, True)
    if debug:
        dbg_kvps = nc.declare_dram_parameter("dbg_kvps", [2, 128, 129], FP32, True)
        dbg_kvblk = nc.declare_dram_parameter("dbg_kvblk", [2, 128, 128], BF16, True)
        dbg_be = nc.declare_dram_parameter("dbg_be", [2, 128, 1], FP32, True)

    PIECE = N // NPIECE

    with TileContext(nc) as tc:
        with (
            tc.tile_pool(name="const", bufs=1) as const,
            tc.tile_pool(name="resident", bufs=1) as resident,
        ):
            # --- resident tensors -------------------------------------------
            x8 = resident.tile([128, 2, N], FP8, name="x8")
            xf = [resident.tile([128, N], FP32, name=f"xf{t}") for t in range(2)]
            wkv8 = const.tile([128, 2, 512], FP8, name="wkv8")
            wqt = [const.tile([128, 256], BF16, name=f"wqt{t}") for t in range(2)]
            wp = [const.tile([128, 256], BF16, name=f"wp{t}") for t in range(2)]
            bq = [const.tile([128, 1], BF16, name=f"bq{t}") for t in range(2)]
            bp = [const.tile([128, 1], FP32, name=f"bp{t}") for t in range(2)]
            bv = [const.tile([128, 32], FP32, name=f"bv{t}") for t in range(2)]
            kvblk = [const.tile([128, 128], BF16, name=f"kvblk{t}") for t in range(2)]
            Gp = [
                [const.tile([128, 128], BF16, name=f"Gp{t}{kc}") for kc in range(2)]
                for t in range(2)
            ]
            M8 = [const.tile([128, 2, 128], FP8, name=f"M8{mt}") for mt in range(2)]
            cq = [const.tile([128, 1], BF16, name=f"cq{t}") for t in range(2)]
            be = [const.tile([128, 1], FP32, name=f"be{mt}") for mt in range(2)]
            recip = [const.tile([128, 1], FP32, name=f"recip{t}") for t in range(2)]
            vb = [const.tile([128, 516], BF16, name=f"vb{j}") for j in range(3)]
            kvsum = const.tile([128, 258], FP32, name="kvsum")

            # phase-1 gates first: x8 (piecewise so chunk 0 starts asap) + wkv8
            nc.sync.dma_start(x8[:, :, 0 : N // 8], x8_d[:, :, 0 : N // 8])
            nc.sync.dma_start(wkv8[:], wkv8_d[:, :, :])
            for i in range(1, 8):
                nc.sync.dma_start(
                    x8[:, :, i * N // 8 : (i + 1) * N // 8],
                    x8_d[:, :, i * N // 8 : (i + 1) * N // 8],
                )
            for t in range(2):
                nc.sync.dma_start(wqt[t][:], wqt_d[t])
                nc.sync.dma_start(wp[t][:], wp_d[t])
                nc.sync.dma_start(bq[t][:], bq_d[t])
                nc.sync.dma_start(bp[t][:], bp_d[t])
                nc.sync.dma_start(bv[t][:], bv_d[t])
                nc.vector.memset(kvblk[t][:], 0.0)
            for j in range(3):
                nc.vector.memset(
                    vb[j][:].rearrange("p (s x) -> p s x", x=129)[:, :, 128:129], 1.0
                )
            # xf only matters from phase 2 on; stream it during phase 1
            for i in range(NPIECE):
                for t in range(2):
                    nc.sync.dma_start(
                        xf[t][:, i * PIECE : (i + 1) * PIECE],
                        xf_d[t, :, i * PIECE : (i + 1) * PIECE],
                    )

            # --- phase 1: k||v, exp, kv accumulation ------------------------
            with (
                tc.tile_pool(name="p1ps", bufs=1, space="PSUM") as p1ps,
                tc.tile_pool(name="kvp_ps", bufs=3, space="PSUM") as kvp_ps,
                tc.tile_pool(name="ework", bufs=4) as ework,
            ):
                # two parity-alternating accumulators (t0 at cols 0:129, t1 at
                # 129:258) so consecutive pairs' kv matmuls are independent
                kvps = [
                    p1ps.tile([128, 258], FP32, name=f"kvps{par}") for par in range(2)
                ]

                for pi in range(NPAIR):
                    par = pi % 2
                    first, last = pi < 2, pi >= NPAIR - 2
                    kvp = kvp_ps.tile([128, 1024], FP32, name="kvp", tag="kvp")
                    for half in range(2):
                        n0 = (pi * 2 + half) * 128
                        f0 = half * 512
                        nc.tensor.matmul(
                            kvp[:, f0 : f0 + 512],
                            lhsT=x8[:, :, n0 : n0 + 128], rhs=wkv8[:],
                            start=True, stop=True, perf_mode=DR,
                        )
                    # one exp over both chunks' k columns (strided view)
                    E = ework.tile([128, 512], BF16, name="E", tag="E")
                    nc.scalar.activation(
                        E[:].rearrange("p (s x) -> p s x", x=256),
                        kvp[:].rearrange("p (s x) -> p s x", x=512)[:, :, 0:256],
                        AF.Exp,
                    )
                    v = vb[pi % 3]
                    nc.vector.tensor_copy(
                        v[:].rearrange("p (h t x) -> p h t x", t=2, x=129)[
                            :, :, :, 0:128
                        ],
                        kvp[:]
                        .rearrange("p (h x) -> p h x", x=512)[:, :, 256:512]
                        .rearrange("p h (t c) -> p h t c", c=128),
                    )
                    for half in range(2):
                        for t in range(2):
                            sec = half * 2 + t
                            nc.tensor.matmul(
                                kvps[par][:, t * 129 : t * 129 + 129],
                                lhsT=E[
                                    :,
                                    half * 256 + t * 128 : half * 256 + t * 128 + 128,
                                ],
                                rhs=v[:, sec * 129 : sec * 129 + 129],
                                start=(first and half == 0),
                                stop=(last and half == 1),
                                skip_group_check=True,
                            )

                # --- finalize kv: merge parities, normalize, add v bias -----
                nc.vector.tensor_copy(kvsum[:], kvps[0][:])
                nc.vector.tensor_add(kvsum[:], kvsum[:], kvps[1][:])
                if debug:
                    for t in range(2):
                        nc.sync.dma_start(
                            dbg_kvps[t], kvsum[:, t * 129 : t * 129 + 129]
                        )
                for t in range(2):
                    c0 = t * 129
                    nc.vector.reciprocal(recip[t][:], kvsum[:, c0 + 128 : c0 + 129])
                    for g in range(4):
                        r0 = g * 32
                        nc.vector.scalar_tensor_tensor(
                            kvblk[t][r0 : r0 + 32, r0 : r0 + 32],
                            kvsum[r0 : r0 + 32, c0 + r0 : c0 + r0 + 32],
                            recip[t][r0 : r0 + 32, :],
                            bv[t][r0 : r0 + 32, :],
                            op0=AluOpType.mult,
                            op1=AluOpType.add,
                        )

            # --- fold: G' = kvblk^T Wq^T, M8 = 2^17 G'^T Wp', bias_eff ------
            with tc.tile_pool(name="gps", bufs=4, space="PSUM") as gps:
                for t in range(2):
                    cq_ps = gps.tile([128, 1], FP32, name=f"cqps{t}", tag="little")
                    nc.tensor.matmul(
                        cq_ps[:], lhsT=kvblk[t][:], rhs=bq[t][:],
                        start=True, stop=True,
                    )
                    nc.vector.tensor_copy(cq[t][:], cq_ps[:])
                    for kc in range(2):
                        g_ps = gps.tile([128, 128], FP32, name=f"gps{t}{kc}", tag="big")
                        nc.tensor.matmul(
                            g_ps[:],
                            lhsT=kvblk[t][:],
                            rhs=wqt[t][:, kc * 128 : kc * 128 + 128],
                            start=True, stop=True,
                        )
                        nc.vector.tensor_copy(Gp[t][kc][:], g_ps[:])
                for mt in range(2):
                    be_ps = gps.tile([128, 1], FP32, name=f"beps{mt}", tag="little")
                    for t in range(2):
                        nc.tensor.matmul(
                            be_ps[:],
                            lhsT=wp[t][:, mt * 128 : mt * 128 + 128],
                            rhs=cq[t][:],
                            start=(t == 0), stop=(t == 1),
                        )
                    nc.vector.tensor_add(be[mt][:], be_ps[:], bp[mt][:])
                    for kc in range(2):
                        m_ps = gps.tile([128, 128], FP32, name=f"mps{kc}{mt}", tag="big")
                        for t in range(2):
                            nc.tensor.matmul(
                                m_ps[:],
                                lhsT=Gp[t][kc][:],
                                rhs=wp[t][:, mt * 128 : mt * 128 + 128],
                                start=(t == 0), stop=(t == 1),
                            )
                        nc.scalar.activation(
                            M8[mt][:, kc, :], m_ps[:], AF.Identity, scale=M_SCALE
                        )
                if debug:
                    for mt in range(2):
                        nc.sync.dma_start(dbg_be[mt], be[mt][:])
                    for t in range(2):
                        nc.sync.dma_start(dbg_kvblk[t], kvblk[t][:])

            # --- phase 2: pp = M8^T x8;  out = pp/2^17 + bias_eff + xf ------
            with (
                tc.tile_pool(name="pp_ps", bufs=6, space="PSUM") as pp_ps,
                tc.tile_pool(name="p2out", bufs=10) as p2out,
            ):
                for cj in range(NJUMBO):
                    n0 = cj * 512
                    for mt in range(2):
                        pp = pp_ps.tile([128, 512], FP32, name="pp", tag="pp")
                        nc.tensor.matmul(
                            pp[:], lhsT=M8[mt][:], rhs=x8[:, :, n0 : n0 + 512],
                            start=True, stop=True, perf_mode=DR,
                        )
                        osb = p2out.tile([128, 512], FP32, name="osb", tag="osb")
                        if mt == 0:
                            tmp = p2out.tile(
                                [128, 512], FP32, name="tmp", tag="tmp"
                            )
                            nc.vector.tensor_scalar(
                                tmp[:], pp[:], 1.0 / M_SCALE, be[mt][:],
                                op0=AluOpType.mult, op1=AluOpType.add,
                            )
                            nc.vector.tensor_add(
                                osb[:], tmp[:], xf[mt][:, n0 : n0 + 512]
                            )
                        else:
                            tmp = p2out.tile(
                                [128, 512], FP32, name="tmp", tag="tmp"
                            )
                            nc.scalar.activation(
                                tmp[:], pp[:], AF.Identity,
                                bias=be[mt][:], scale=1.0 / M_SCALE,
                            )
                            nc.gpsimd.tensor_add(
                                osb[:], tmp[:], xf[mt][:, n0 : n0 + 512]
                            )
                        nc.sync.dma_start(out_d[mt, :, n0 : n0 + 512], osb[:])
    nc.finalize()
    return nc


def _get_nc():
    if "nc" not in _CACHE:
        _CACHE["nc"] = _build_nc()
    return _CACHE["nc"]


def _prep_in_maps(x, W_qkv, b_qkv, W_proj, b_proj, gamma):
    bf = ml_dtypes.bfloat16
    f8 = ml_dtypes.float8_e4m3
    scale = 32 ** (-0.5)
    g = float(np.asarray(gamma).reshape(-1)[0])

    # fp8 operands use contraction index c = ko*128 + ki -> layout [ki, ko, :]
    Wkv8 = np.ascontiguousarray(
        W_qkv[:, 256:768].reshape(2, 128, 512).swapaxes(0, 1)).astype(f8)
    WqT = np.ascontiguousarray(
        W_qkv[:, 0:256].T.reshape(2, 128, 256)).astype(bf)
    Wp = np.ascontiguousarray(
        (W_proj * (scale * g)).reshape(2, 128, 256)).astype(bf)
    bq = np.ascontiguousarray(
        b_qkv[0:256].reshape(2, 128, 1)).astype(bf)
    bp = np.ascontiguousarray(
        (g * b_proj).reshape(2, 128, 1)).astype(np.float32)
    # bv[t][p, cv] = b_qkv[512 + (t*4 + p//32)*32 + cv]
    bv = np.ascontiguousarray(
        np.broadcast_to(
            b_qkv[512:768].reshape(2, 4, 1, 32), (2, 4, 32, 32)
        ).reshape(2, 128, 32)
    ).astype(np.float32)

    in_maps = []
    for b in range(NCORES):
        xb = np.ascontiguousarray(x[b].reshape(C, N))
        x8 = np.ascontiguousarray(
            xb.reshape(2, 128, N).swapaxes(0, 1)).astype(f8)
        in_maps.append(
            {
                "x8": x8,
                "xf": xb.reshape(2, 128, N),
                "wkv8": Wkv8, "wqt": WqT, "wp": Wp,
                "bq": bq, "bp": bp, "bv": bv,
            }
        )
    return in_maps


def kernel(x, W_qkv, b_qkv, W_proj, b_proj, gamma, _trace=False, _trace_kwargs=None):
    x = np.asarray(x, dtype=np.float32)
    nc = _get_nc()
    in_maps = _prep_in_maps(
        x,
        np.asarray(W_qkv, np.float32),
        np.asarray(b_qkv, np.float32),
        np.asarray(W_proj, np.float32),
        np.asarray(b_proj, np.float32),
        np.asarray(gamma, np.float32),
    )
    kw = {}
    if _trace:
        kw = {"trace": True, **(_trace_kwargs or {})}
    res = run_bass_kernel_spmd(nc, in_maps, list(range(NCORES)), **kw)
    out = np.stack(
        [res.results[b]["out"].reshape(C, 3, 64, 64) for b in range(NCORES)]
    ).astype(np.float32)
    if _trace:
        return out, res
    return out
